# revision 3
# baseline (speedup 1.0000x reference)
"""Trainium2 Bass kernel: 15x15 valid cross-correlation of a 4096x4096 image.

Strategy
--------
out[h, w] = sum_{kh, kw} x[h+kh, w+kw] * wt[kh, kw]  (+ bias)

Sharding: output columns are split across the 8 NeuronCores (512 out cols per
core; each core reads a [4096, 526] column slab of x — a 14-col halo).

Per core, the conv is computed as banded-Toeplitz matmuls on the TensorEngine:
for each block of M=114 output rows (K = M+14 = 128 input rows on the
partition dim) and each kw in [0, 15):

    psum[:M, :512] += T_kw.T @ x_blk[:, kw : kw+512]

where T_kw[h_in, h_out] = wt[h_in - h_out, kw] is the kh-band. The kw tap is a
free-dim offset into the same SBUF tile, so no data duplication is needed.
The 15 Toeplitz matrices are built host-side and passed as one [128, 15*114]
input. Matmuls run as float32r (full-rate fp32 streaming), accumulating fp32
in PSUM; the bias is fused into the PSUM->SBUF drain on the VectorEngine.
"""

import numpy as np

H = 4096
W = 4096
KH = 15
KW = 15
OH = H - KH + 1  # 4082
OW = W - KW + 1  # 4082
NCORES = 8
COLS = 512              # output cols per core
INC = COLS + KW - 1     # 526 input cols per core
BLK = 114               # output rows per row-block (K = BLK + 14 = 128)
NBLK = (OH + BLK - 1) // BLK  # 36 (last block M=92)

_CACHE = {}


def _build_program():
    import concourse.tile as tile
    from concourse import bacc, mybir
    from contextlib import ExitStack

    nc = bacc.Bacc("TRN2", target_bir_lowering=False, debug=False,
                   num_devices=NCORES)
    f32r = mybir.dt.float32r
    x_d = nc.dram_tensor("x", [H, INC], f32r,
                         kind="ExternalInput").ap()
    w_d = nc.dram_tensor("wt", [128, KW * BLK], f32r,
                         kind="ExternalInput").ap()
    b_d = nc.dram_tensor("bias", [128, 1], mybir.dt.float32,
                         kind="ExternalInput").ap()
    o_d = nc.dram_tensor("out", [OH, COLS], mybir.dt.float32,
                         kind="ExternalOutput").ap()

    with ExitStack() as ctx:
        tc = ctx.enter_context(tile.TileContext(nc))
        wpool = ctx.enter_context(tc.tile_pool(name="wp", bufs=1))
        bpool = ctx.enter_context(tc.tile_pool(name="bp", bufs=1))
        xpool = ctx.enter_context(tc.tile_pool(name="xp", bufs=3))
        opool = ctx.enter_context(tc.tile_pool(name="op", bufs=3))
        pspool = ctx.enter_context(tc.tile_pool(name="ps", bufs=4, space="PSUM"))

        wt_t = wpool.tile([128, KW * BLK], f32r)
        nc.sync.dma_start(wt_t[:], w_d[:])
        b_t = bpool.tile([128, 1], mybir.dt.float32)
        nc.sync.dma_start(b_t[:], b_d[:])

        for b in range(NBLK):
            r0 = b * BLK
            m = min(BLK, OH - r0)
            k = m + KH - 1
            x_t = xpool.tile([128, INC], f32r)
            nc.sync.dma_start(x_t[:k, :], x_d[r0:r0 + k, :])
            ps = pspool.tile([BLK, COLS], mybir.dt.float32)
            for kw in range(KW):
                nc.tensor.matmul(
                    ps[:m, :],
                    wt_t[:k, kw * BLK: kw * BLK + m],
                    x_t[:k, kw: kw + COLS],
                    start=(kw == 0),
                    stop=(kw == KW - 1),
                )
            o_t = opool.tile([BLK, COLS], mybir.dt.float32)
            nc.vector.tensor_scalar_add(o_t[:m, :], ps[:m, :], b_t[:m, :])
            nc.sync.dma_start(o_d[r0:r0 + m, :], o_t[:m, :])

    nc.compile()
    return nc


def _toeplitz(weight):
    wtoep = np.zeros((128, KW * BLK), np.float32)
    idx = np.arange(BLK)
    for kw in range(KW):
        for d in range(KH):  # d = h_in - h_out
            wtoep[idx + d, kw * BLK + idx] = weight[d, kw]
    return wtoep


def _prepare_in_maps(x, weight, bias):
    x = np.asarray(x, dtype=np.float32)
    weight = np.asarray(weight, dtype=np.float32)
    bias = np.asarray(bias, dtype=np.float32)

    x_pad = np.zeros((H, NCORES * COLS + KW - 1), np.float32)
    x_pad[:, :W] = x
    wtoep = _toeplitz(weight)
    bias_b = np.full((128, 1), bias.reshape(-1)[0], np.float32)

    in_maps = []
    for c in range(NCORES):
        shard = np.ascontiguousarray(x_pad[:, c * COLS: c * COLS + INC])
        in_maps.append({"x": shard, "wt": wtoep, "bias": bias_b})
    return in_maps


def _run(x, weight, bias, trace=False):
    from concourse.bass_utils import run_bass_kernel_spmd

    if "nc" not in _CACHE:
        _CACHE["nc"] = _build_program()
    nc = _CACHE["nc"]

    in_maps = _prepare_in_maps(x, weight, bias)
    res = run_bass_kernel_spmd(nc, in_maps, core_ids=list(range(NCORES)),
                               trace=trace)
    out = np.empty((OH, NCORES * COLS), np.float32)
    for c in range(NCORES):
        out[:, c * COLS: (c + 1) * COLS] = res.results[c]["out"]
    return out[:, :OW], res


def kernel(x, weight, bias):
    out, _ = _run(x, weight, bias, trace=False)
    return out
